# Initial kernel scaffold
#
"""
Trainium2 Bass kernel for the ConnSNN module (33-step spiking neural net scan).

Strategy (data-parallel over batch, 8 cores x 1024 rows):
  - All state kept on-chip in transposed [neuron, batch] layout so the
    recurrent spike matmul needs no transposes: psum[n,b] += w_hs[j,n]*S[j,b]
    with the (constant) hidden weights as the stationary operand.
  - Spikes S in {0,1} fp8; inhibitory sign folded into the weights
    (w_hs = +-w_h), so the matmul stays exact integer arithmetic.
  - Elementwise recurrences restructured into fused scalar_tensor_tensor ops
    on DVE/GPSIMD with all state rescaled by 1/R4 so the i_syn update reads
    PSUM directly:   D' = AS*D + P        (D = i_syn/R_H)
                     v' = AV*v + D'       (v~ = (v_m - i_in)/R4)
                     S  = v~' > thr       (thr = (1-i_in)/R4)
                     reset via copy_predicated(v~, S, negI)  (negI = -i_in/R4)
                     rho' = rho + c_t*S   (rho = rate/AO^t, sign per tile)
  - i_in computed once via a hi/lo-split bf16 matmul of x.T against
    w_diff = w_in[:512] - w_in[512:]  (exact f32-class accuracy; the spike
    threshold makes the system chaotic, so bf16/fp22-class errors in the
    dynamics are fatal while ~1e-6 reorder-class noise is the floor).
  - Final readout matmul in f32r (fp22) — output-only, no feedback.
"""

import math

import ml_dtypes
import numpy as np

import concourse.bass as bass
import concourse.mybir as mybir
import concourse.tile as tile
from concourse.bass_utils import run_bass_kernel_spmd
from concourse.vector_clock import ScopedClock

# ---- module constants ----
N = 1024          # neurons
IN = 512          # input dims
OUT = 128         # output dims
NE = 512          # excitatory count
NSTEPS = 33
B = 8192          # full batch
NCORES = 8
BL = B // NCORES  # rows per core (1024)
F = BL            # on-chip free dim

R_IN = 0.1 * 1.0 * 10.0 * math.sqrt(2.0 / (0.5 * IN))
R_H = 1.0 * 1.0 * 10.0 / 5.0 * math.sqrt(2.0 / (0.5 * N))
R_OUT = 5.0 * math.sqrt(1.0 / (0.5 * N))
AS = math.exp(-0.5 / 5.0)
AO = math.exp(-0.5 / 10.0)
AV = math.exp(-0.5 / 10.0)
R4 = (1.0 - AV) * R_H

dt = mybir.dt
F32, BF16, I32 = dt.float32, dt.bfloat16, dt.int32
FP8 = dt.float8e4
F32R = dt.float32r
NP_BF16 = ml_dtypes.bfloat16
NP_FP8 = dt.np(FP8)

# tiles whose elementwise chain (v-update, threshold, rate) runs on GPSIMD
# instead of DVE; DVE always runs the psum-reading i_syn update and the reset.
POOL_TILES = frozenset({2, 3, 4, 5, 6, 7})

OP = mybir.AluOpType

_patched = False


def _patch_tile_drain():
    """This walrus build rejects instructions carrying >2 sync waits; Tile's
    exit barrier puts every outstanding sem wait on one SP drain. Spread the
    waits over single-wait nops instead."""
    global _patched
    if _patched:
        return
    _patched = True

    def patched(self, tick_clock, wait_clock):
        nc = self.nc
        drain_inst = nc.sync.drain()
        wait_clock.add_sem_waits(
            drain_inst.ins, ScopedClock({None: tick_clock.global_clock})
        )
        si = drain_inst.ins.sync_info
        if si is not None and len(si.on_wait) > 1:
            waits = list(si.on_wait)
            drain_inst.ins.sync_info = mybir.SyncInfo(
                on_wait=[], on_update=list(si.on_update)
            )
            for w in waits:
                nop = nc.sync.nop()
                nop.ins.sync_info = mybir.SyncInfo(on_wait=[w], on_update=[])
        nc.all_engine_barrier()
        popped = nc._tile_sem_poison_stack.pop()
        assert popped is self._sem_poison
        nc.clear_and_free_semaphores(list(self.sems.allocated().values()))
        nc.all_engine_barrier()

    tile.TileContext._drain_and_barrier = patched


_nc_cache = None


def _build_nc():
    global _nc_cache
    if _nc_cache is not None:
        return _nc_cache
    _patch_tile_drain()

    nc = bass.Bass()
    P = nc.declare_dram_parameter
    wdiff_d = P("wdiff", [IN, N], BF16, isOutput=False)
    xhi_d = P("xhi", [IN, F], BF16, isOutput=False)
    xlo_d = P("xlo", [IN, F], BF16, isOutput=False)
    whs_d = P("whs", [N, N], FP8, isOutput=False)
    wout_d = P("wout", [N, OUT], F32, isOutput=False)
    vT_d = P("vT", [N, F], F32, isOutput=False)
    isynT_d = P("isynT", [N, F], F32, isOutput=False)
    rateT_d = P("rateT", [N, F], F32, isOutput=False)
    spkT_d = P("spkT", [N, F], FP8, isOutput=False)

    v_o = P("v_o", [N, F], F32, isOutput=True)
    isyn_o = P("isyn_o", [N, F], F32, isOutput=True)
    rate_o = P("rate_o", [N, F], F32, isOutput=True)
    spk_o = P("spk_o", [N, F], I32, isOutput=True)
    out_o = P("out_o", [OUT, F], F32, isOutput=True)

    dma = nc.gpsimd.dma_start

    with tile.TileContext(nc) as tc:
        with (
            tc.tile_pool(name="state", bufs=1) as st,
            tc.tile_pool(name="spk", bufs=1) as sp,
            tc.tile_pool(name="wh", bufs=1) as wh,
        ):
            vt = [st.tile([128, F], F32, tag=f"vt{i}") for i in range(8)]
            Dt = [st.tile([128, F], F32, tag=f"d{i}") for i in range(8)]
            rho = [st.tile([128, F], F32, tag=f"r{i}") for i in range(8)]
            negI = [st.tile([128, F], F32, tag=f"n{i}") for i in range(8)]
            S = [
                [sp.tile([128, F], FP8, tag=f"s{p}_{i}") for i in range(8)]
                for p in range(2)
            ]
            whs_t = [wh.tile([128, N], FP8, tag=f"w{i}") for i in range(8)]

            for i in range(8):
                sl = slice(128 * i, 128 * (i + 1))
                dma(Dt[i][:], isynT_d[sl, :])
                dma(vt[i][:], vT_d[sl, :])
                dma(rho[i][:], rateT_d[sl, :])
                dma(S[0][i][:], spkT_d[sl, :])
                dma(whs_t[i][:], whs_d[sl, :])

            with tc.tile_pool(name="thr", bufs=1) as thp:
                thr = [thp.tile([128, F], F32, tag=f"t{i}") for i in range(8)]

                # ---- phase A: i_in = R_IN * (w_diff.T @ x.T), hi/lo split ----
                with (
                    tc.tile_pool(name="wd", bufs=1) as wdp,
                    tc.tile_pool(name="xin", bufs=6) as xp,
                    tc.tile_pool(name="psA", bufs=8, space="PSUM") as ppA,
                ):
                    wd = [wdp.tile([128, N], BF16, tag=f"wd{i}") for i in range(4)]
                    for kc in range(4):
                        dma(wd[kc][:], wdiff_d[128 * kc : 128 * (kc + 1), :])
                    for bc in range(2):
                        bsl = slice(512 * bc, 512 * (bc + 1))
                        psA = [ppA.tile([128, 512], F32, tag="psa") for _ in range(8)]
                        for kc in range(4):
                            ksl = slice(128 * kc, 128 * (kc + 1))
                            for part, src in ((0, xhi_d), (1, xlo_d)):
                                xt = xp.tile([128, 512], BF16, tag="xc")
                                dma(xt[:], src[ksl, bsl])
                                for nt in range(8):
                                    nsl = slice(128 * nt, 128 * (nt + 1))
                                    nc.tensor.matmul(
                                        psA[nt][:],
                                        wd[kc][:, nsl],
                                        xt[:],
                                        start=(kc == 0 and part == 0),
                                        stop=(kc == 3 and part == 1),
                                        skip_group_check=True,
                                    )
                        for nt in range(8):
                            nc.scalar.mul(negI[nt][:, bsl], psA[nt][:], -R_IN / R4)

                # derived constants + state rescale (raw units)
                for nt in range(8):
                    nc.vector.tensor_scalar_add(thr[nt][:], negI[nt][:], 1.0 / R4)
                    nc.vector.scalar_tensor_tensor(
                        vt[nt][:], vt[nt][:], 1.0 / R4, negI[nt][:], OP.mult, OP.add
                    )
                    nc.vector.tensor_scalar_mul(Dt[nt][:], Dt[nt][:], 1.0 / R_H)

                # ---- phase B: the 33-step scan ----
                with tc.tile_pool(name="psB", bufs=4, space="PSUM") as ppB:
                    for t in range(NSTEPS):
                        Scur = S[t % 2]
                        Snew = S[(t + 1) % 2]
                        ct0 = 2.0 * (1.0 - AO) / (AO ** (t + 1))
                        pslist = []
                        for nt in range(8):
                            nsl = slice(128 * nt, 128 * (nt + 1))
                            ps = ppB.tile([128, F], F32, tag="ps")
                            pslist.append(ps)
                            for bc in range(2):
                                bsl = slice(512 * bc, 512 * (bc + 1))
                                for j in range(8):
                                    nc.tensor.matmul(
                                        ps[:, bsl],
                                        whs_t[j][:, nsl],
                                        Scur[j][:, bsl],
                                        start=(j == 0),
                                        stop=(j == 7),
                                    )
                        for nt in range(8):
                            eng = nc.gpsimd if nt in POOL_TILES else nc.vector
                            nc.vector.scalar_tensor_tensor(
                                Dt[nt][:], Dt[nt][:], AS, pslist[nt][:], OP.mult, OP.add
                            )
                            eng.scalar_tensor_tensor(
                                vt[nt][:], vt[nt][:], AV, Dt[nt][:], OP.mult, OP.add
                            )
                            eng.tensor_tensor(
                                Snew[nt][:], vt[nt][:], thr[nt][:], OP.is_gt
                            )
                            nc.vector.copy_predicated(
                                vt[nt][:], Snew[nt][:], negI[nt][:]
                            )
                            ct = ct0 if nt < 4 else -ct0
                            eng.scalar_tensor_tensor(
                                rho[nt][:], Snew[nt][:], ct, rho[nt][:], OP.mult, OP.add
                            )

                    # ---- phase C: unscale, emit outputs, readout matmul ----
                    Sfin = S[NSTEPS % 2]
                    with (
                        tc.tile_pool(name="wo", bufs=1) as wop,
                        tc.tile_pool(name="cout", bufs=3) as cp,
                    ):
                        for nt in range(8):
                            sl = slice(128 * nt, 128 * (nt + 1))
                            si = cp.tile([128, F], I32, tag="si")
                            nc.vector.tensor_scalar_mul(
                                si[:], Sfin[nt][:], 1.0 if nt < 4 else -1.0
                            )
                            dma(spk_o[sl, :], si[:])
                            nc.vector.tensor_scalar_mul(Dt[nt][:], Dt[nt][:], R_H)
                            dma(isyn_o[sl, :], Dt[nt][:])
                            nc.vector.tensor_sub(vt[nt][:], vt[nt][:], negI[nt][:])
                            nc.vector.tensor_scalar_mul(vt[nt][:], vt[nt][:], R4)
                            dma(v_o[sl, :], vt[nt][:])
                            nc.vector.tensor_scalar_mul(
                                rho[nt][:], rho[nt][:], AO**NSTEPS
                            )
                            dma(rate_o[sl, :], rho[nt][:])

                        wo = [wop.tile([128, OUT], F32, tag=f"wo{i}") for i in range(8)]
                        for j in range(8):
                            dma(wo[j][:], wout_d[128 * j : 128 * (j + 1), :])
                        ps = ppB.tile([128, F], F32, tag="ps")
                        for bc in range(2):
                            bsl = slice(512 * bc, 512 * (bc + 1))
                            for j in range(8):
                                nc.tensor.matmul(
                                    ps[:, bsl],
                                    wo[j][:].bitcast(F32R),
                                    rho[j][:, bsl].bitcast(F32R),
                                    start=(j == 0),
                                    stop=(j == 7),
                                )
                        ot = cp.tile([OUT, F], F32, tag="ot")
                        nc.scalar.mul(ot[:], ps[:], R_OUT)
                        dma(out_o[:, :], ot[:])

    _nc_cache = nc
    return nc


def _prep_in_maps(x, v_m, i_syn, rate, spike):
    f32 = np.float32
    sign = np.ones((N, 1), f32)
    sign[NE:] = -1.0
    wdiff = _prep_in_maps.wdiff
    whs = _prep_in_maps.whs
    wout = _prep_in_maps.wout
    in_maps = []
    for c in range(NCORES):
        rows = slice(BL * c, BL * (c + 1))
        xt = np.ascontiguousarray(x[rows].T).astype(f32)
        xhi = xt.astype(NP_BF16)
        xlo = (xt - xhi.astype(f32)).astype(NP_BF16)
        in_maps.append(
            dict(
                wdiff=wdiff,
                xhi=xhi,
                xlo=xlo,
                whs=whs,
                wout=wout,
                vT=np.ascontiguousarray(v_m[rows].T).astype(f32),
                isynT=np.ascontiguousarray(i_syn[rows].T).astype(f32),
                rateT=np.ascontiguousarray(rate[rows].T).astype(f32),
                spkT=np.ascontiguousarray(np.abs(spike[rows]).T).astype(NP_FP8),
            )
        )
    return in_maps


def _run(x, v_m, i_syn, rate, spike, kernel_in, kernel_h, kernel_out, **runkw):
    f32 = np.float32
    ki = np.asarray(kernel_in)
    kh = np.asarray(kernel_h)
    ko = np.asarray(kernel_out)
    sign = np.ones((N, 1), f32)
    sign[NE:] = -1.0
    _prep_in_maps.wdiff = np.ascontiguousarray(
        ki[:IN].astype(np.float32) - ki[IN:].astype(np.float32)
    ).astype(NP_BF16)
    _prep_in_maps.whs = np.ascontiguousarray(kh.astype(f32) * sign).astype(NP_FP8)
    _prep_in_maps.wout = np.ascontiguousarray(ko).astype(f32)

    in_maps = _prep_in_maps(
        np.asarray(x), np.asarray(v_m), np.asarray(i_syn),
        np.asarray(rate), np.asarray(spike),
    )
    nc = _build_nc()
    res = run_bass_kernel_spmd(nc, in_maps, list(range(NCORES)), **runkw)

    f32 = np.float32
    v = np.concatenate([res.results[c]["v_o"].T for c in range(NCORES)], axis=0)
    isyn = np.concatenate([res.results[c]["isyn_o"].T for c in range(NCORES)], axis=0)
    rate_f = np.concatenate([res.results[c]["rate_o"].T for c in range(NCORES)], axis=0)
    spk = np.concatenate([res.results[c]["spk_o"].T for c in range(NCORES)], axis=0)
    out = np.concatenate([res.results[c]["out_o"].T for c in range(NCORES)], axis=0)
    outputs = (
        v.astype(f32),
        isyn.astype(f32),
        rate_f.astype(f32),
        spk.astype(np.int32),
        out.astype(f32),
    )
    return outputs, res


def kernel(x, v_m, i_syn, rate, spike, kernel_in, kernel_h, kernel_out):
    outputs, _ = _run(x, v_m, i_syn, rate, spike, kernel_in, kernel_h, kernel_out)
    return outputs


# revision 13
# speedup vs baseline: 61.9102x; 61.9102x over previous
"""
Trainium2 Bass kernel for the ConnSNN module (33-step spiking neural net scan).

Strategy (data-parallel over batch, 8 cores x 1024 rows):
  - All state kept on-chip in transposed [neuron, batch] layout so the
    recurrent spike matmul needs no transposes: psum[n,b] += w_hs[j,n]*S[j,b]
    with the (constant) hidden weights as the stationary operand.
  - Spikes S in {0,1} fp8; inhibitory sign folded into the weights
    (w_hs = +-w_h), so the matmul stays exact integer arithmetic.
  - Elementwise recurrences restructured into fused scalar_tensor_tensor ops
    on DVE/GPSIMD with all state rescaled by 1/R4 so the i_syn update reads
    PSUM directly:   D' = AS*D + P        (D = i_syn/R_H)
                     v' = AV*v + D'       (v~ = (v_m - i_in)/R4)
                     S  = v~' > thr       (thr = (1-i_in)/R4)
                     reset via copy_predicated(v~, S, negI)  (negI = -i_in/R4)
                     rho' = rho + c_t*S   (rho = rate/AO^t, sign per tile)
  - i_in computed once via a hi/lo-split bf16 matmul of x.T against
    w_diff = w_in[:512] - w_in[512:]  (exact f32-class accuracy; the spike
    threshold makes the system chaotic, so bf16/fp22-class errors in the
    dynamics are fatal while ~1e-6 reorder-class noise is the floor).
  - Final readout matmul in f32r (fp22) — output-only, no feedback.
"""

import math

import ml_dtypes
import numpy as np

import concourse.bass as bass
import concourse.mybir as mybir
import concourse.tile as tile
from concourse.bass_utils import run_bass_kernel_spmd
from concourse.vector_clock import ScopedClock

# ---- module constants ----
N = 1024          # neurons
IN = 512          # input dims
OUT = 128         # output dims
NE = 512          # excitatory count
NSTEPS = 33
B = 8192          # full batch
NCORES = 8
BL = B // NCORES  # rows per core (1024)
F = BL            # on-chip free dim

R_IN = 0.1 * 1.0 * 10.0 * math.sqrt(2.0 / (0.5 * IN))
R_H = 1.0 * 1.0 * 10.0 / 5.0 * math.sqrt(2.0 / (0.5 * N))
R_OUT = 5.0 * math.sqrt(1.0 / (0.5 * N))
AS = math.exp(-0.5 / 5.0)
AO = math.exp(-0.5 / 10.0)
AV = math.exp(-0.5 / 10.0)
R4 = (1.0 - AV) * R_H

dt = mybir.dt
F32, BF16, I32 = dt.float32, dt.bfloat16, dt.int32
FP8 = dt.float8e4
F32R = dt.float32r
NP_BF16 = ml_dtypes.bfloat16
NP_FP8 = dt.np(FP8)

# tiles whose elementwise chain (v-update, threshold, rate) runs on GPSIMD
# instead of DVE; DVE always runs the psum-reading i_syn update and the reset.
POOL_TILES = frozenset({2, 3, 4, 5, 6, 7})

OP = mybir.AluOpType

_patched = False


def _patch_tile_drain():
    """This walrus build rejects instructions carrying >2 sync waits; Tile's
    exit barrier puts every outstanding sem wait on one SP drain. Spread the
    waits over single-wait nops instead."""
    global _patched
    if _patched:
        return
    _patched = True

    def patched(self, tick_clock, wait_clock):
        nc = self.nc
        drain_inst = nc.sync.drain()
        wait_clock.add_sem_waits(
            drain_inst.ins, ScopedClock({None: tick_clock.global_clock})
        )
        si = drain_inst.ins.sync_info
        if si is not None and len(si.on_wait) > 1:
            waits = list(si.on_wait)
            drain_inst.ins.sync_info = mybir.SyncInfo(
                on_wait=[], on_update=list(si.on_update)
            )
            for w in waits:
                nop = nc.sync.nop()
                nop.ins.sync_info = mybir.SyncInfo(on_wait=[w], on_update=[])
        nc.all_engine_barrier()
        popped = nc._tile_sem_poison_stack.pop()
        assert popped is self._sem_poison
        nc.clear_and_free_semaphores(list(self.sems.allocated().values()))
        nc.all_engine_barrier()

    tile.TileContext._drain_and_barrier = patched


def _split_excess_waits(nc):
    """This walrus build allows very few sync-wait commands per instruction
    (DMA pseudo-instructions appear to allow just one). Move every wait of a
    multi-wait instruction onto its own same-engine NoOp placed right before
    it — engines are in-order, so semantics are identical."""
    for f in nc.m.functions:
        for blk in f.blocks:
            insts = blk.instructions
            i = 0
            new_list = []
            changed = False
            for inst in insts:
                si = getattr(inst, "sync_info", None)
                waits = list(si.on_wait) if si is not None else []
                if len(waits) > 1:
                    changed = True
                    for w in waits:
                        nop = mybir.InstNoOp(
                            name=f"{inst.name}-w{i}", ins=[], outs=[]
                        )
                        i += 1
                        nop.engine = inst.engine
                        nop.sync_info = mybir.SyncInfo(on_wait=[w], on_update=[])
                        new_list.append(nop)
                    inst.sync_info = mybir.SyncInfo(
                        on_wait=[], on_update=list(si.on_update)
                    )
                new_list.append(inst)
            if changed:
                try:
                    blk.instructions[:] = new_list
                except TypeError:
                    blk.set_instructions(new_list)


_nc_cache = {}


def _build_nc(nsteps=NSTEPS):
    if nsteps in _nc_cache:
        return _nc_cache[nsteps]
    _patch_tile_drain()

    nc = bass.Bass()
    P = nc.declare_dram_parameter
    wdiff_d = P("wdiff", [IN, N], BF16, isOutput=False)
    xhi_d = P("xhi", [IN, F], BF16, isOutput=False)
    xlo_d = P("xlo", [IN, F], BF16, isOutput=False)
    whs_d = P("whs", [N, N], FP8, isOutput=False)
    wout_d = P("wout", [N, OUT], F32, isOutput=False)
    vT_d = P("vT", [N, F], F32, isOutput=False)
    isynT_d = P("isynT", [N, F], F32, isOutput=False)
    rateT_d = P("rateT", [N, F], F32, isOutput=False)
    spkT_d = P("spkT", [N, F], FP8, isOutput=False)

    v_o = P("v_o", [N, F], F32, isOutput=True)
    isyn_o = P("isyn_o", [N, F], F32, isOutput=True)
    rate_o = P("rate_o", [N, F], F32, isOutput=True)
    spk_o = P("spk_o", [N, F], I32, isOutput=True)
    out_o = P("out_o", [OUT, F], F32, isOutput=True)

    dma = nc.gpsimd.dma_start

    with tile.TileContext(nc) as tc:
        with (
            tc.tile_pool(name="state", bufs=1) as st,
            tc.tile_pool(name="spk", bufs=1) as sp,
            tc.tile_pool(name="wh", bufs=1) as wh,
        ):
            vt = [st.tile([128, F], F32, tag=f"vt{i}", name=f"vt{i}") for i in range(8)]
            Dt = [st.tile([128, F], F32, tag=f"d{i}", name=f"d{i}") for i in range(8)]
            rho = [st.tile([128, F], F32, tag=f"r{i}", name=f"r{i}") for i in range(8)]
            negI = [st.tile([128, F], F32, tag=f"n{i}", name=f"n{i}") for i in range(8)]
            S = [
                [sp.tile([128, F], FP8, tag=f"s{p}_{i}", name=f"s{p}_{i}") for i in range(8)]
                for p in range(2)
            ]
            whs_t = [wh.tile([128, N], FP8, tag=f"w{i}", name=f"w{i}") for i in range(8)]

            for i in range(8):
                sl = slice(128 * i, 128 * (i + 1))
                dma(Dt[i][:], isynT_d[sl, :])
                dma(vt[i][:], vT_d[sl, :])
                dma(rho[i][:], rateT_d[sl, :])
                dma(S[0][i][:], spkT_d[sl, :])
                dma(whs_t[i][:], whs_d[sl, :])

            with tc.tile_pool(name="thr", bufs=1) as thp:
                thr = [thp.tile([128, F], F32, tag=f"t{i}", name=f"t{i}") for i in range(8)]

                # ---- phase A: i_in = R_IN * (w_diff.T @ x.T), hi/lo split ----
                with (
                    tc.tile_pool(name="wd", bufs=1) as wdp,
                    tc.tile_pool(name="xin", bufs=6) as xp,
                    tc.tile_pool(name="psA", bufs=8, space="PSUM") as ppA,
                ):
                    wd = [wdp.tile([128, N], BF16, tag=f"wd{i}", name=f"wd{i}") for i in range(4)]
                    for kc in range(4):
                        dma(wd[kc][:], wdiff_d[128 * kc : 128 * (kc + 1), :])
                    for bc in range(2):
                        bsl = slice(512 * bc, 512 * (bc + 1))
                        psA = [ppA.tile([128, 512], F32, tag="psa", name="psa") for _ in range(8)]
                        for kc in range(4):
                            ksl = slice(128 * kc, 128 * (kc + 1))
                            for part, src in ((0, xhi_d), (1, xlo_d)):
                                xt = xp.tile([128, 512], BF16, tag="xc", name="xc")
                                dma(xt[:], src[ksl, bsl])
                                for nt in range(8):
                                    nsl = slice(128 * nt, 128 * (nt + 1))
                                    nc.tensor.matmul(
                                        psA[nt][:],
                                        wd[kc][:, nsl],
                                        xt[:],
                                        start=(kc == 0 and part == 0),
                                        stop=(kc == 3 and part == 1),
                                        skip_group_check=True,
                                    )
                        for nt in range(8):
                            nc.scalar.mul(negI[nt][:, bsl], psA[nt][:], -R_IN / R4)

                # derived constants + state rescale (raw units)
                for nt in range(8):
                    nc.vector.tensor_scalar_add(thr[nt][:], negI[nt][:], 1.0 / R4)
                    nc.vector.scalar_tensor_tensor(
                        vt[nt][:], vt[nt][:], 1.0 / R4, negI[nt][:], OP.mult, OP.add
                    )
                    nc.vector.tensor_scalar_mul(Dt[nt][:], Dt[nt][:], 1.0 / R_H)

                # ---- phase B: the 33-step scan ----
                with (
                    tc.tile_pool(name="psB", bufs=4, space="PSUM") as ppB,
                    tc.tile_pool(name="btmp", bufs=2) as btp,
                ):
                    for t in range(nsteps):
                        Scur = S[t % 2]
                        Snew = S[(t + 1) % 2]
                        ct0 = 2.0 * (1.0 - AO) / (AO ** (t + 1))
                        pslist = []
                        for nt in range(8):
                            nsl = slice(128 * nt, 128 * (nt + 1))
                            ps = ppB.tile([128, F], F32, tag="ps", name="ps")
                            pslist.append(ps)
                            for bc in range(2):
                                bsl = slice(512 * bc, 512 * (bc + 1))
                                for j in range(8):
                                    nc.tensor.matmul(
                                        ps[:, bsl],
                                        whs_t[j][:, nsl],
                                        Scur[j][:, bsl],
                                        start=(j == 0),
                                        stop=(j == 7),
                                    )
                        for nt in range(8):
                            # D' = AS*D + P   (DVE, reads PSUM directly)
                            nc.vector.scalar_tensor_tensor(
                                Dt[nt][:], Dt[nt][:], AS, pslist[nt][:], OP.mult, OP.add
                            )
                            # v' = AV*v + D'  (scale on ACT, add on POOL)
                            nc.scalar.mul(vt[nt][:], vt[nt][:], AV)
                            nc.gpsimd.tensor_tensor(
                                vt[nt][:], vt[nt][:], Dt[nt][:], OP.add
                            )
                            # S = v' > thr    (DVE)
                            nc.vector.tensor_tensor(
                                Snew[nt][:], vt[nt][:], thr[nt][:], OP.is_gt
                            )
                            # reset: v'[S] = -i_in  (DVE)
                            nc.vector.copy_predicated(
                                vt[nt][:], Snew[nt][:].bitcast(dt.uint8), negI[nt][:]
                            )
                            # rho += ct*S  (scale on ACT, add on POOL/DVE)
                            ct = ct0 if nt < 4 else -ct0
                            rtmp = btp.tile([128, F], BF16, tag="rtmp", name="rtmp")
                            nc.scalar.mul(rtmp[:], Snew[nt][:], ct)
                            radd_eng = nc.gpsimd if nt in POOL_TILES else nc.vector
                            radd_eng.tensor_tensor(
                                rho[nt][:], rho[nt][:], rtmp[:], OP.add
                            )

                    # ---- phase C: unscale, emit outputs, readout matmul ----
                    Sfin = S[nsteps % 2]
                    with (
                        tc.tile_pool(name="wo", bufs=1) as wop,
                        tc.tile_pool(name="cout", bufs=2) as cp,
                    ):
                        for nt in range(8):
                            sl = slice(128 * nt, 128 * (nt + 1))
                            si = cp.tile([128, F], I32, tag="si", name="si")
                            nc.vector.tensor_scalar_mul(
                                si[:], Sfin[nt][:], 1.0 if nt < 4 else -1.0
                            )
                            dma(spk_o[sl, :], si[:])
                            nc.vector.tensor_scalar_mul(Dt[nt][:], Dt[nt][:], R_H)
                            dma(isyn_o[sl, :], Dt[nt][:])
                            nc.vector.tensor_sub(vt[nt][:], vt[nt][:], negI[nt][:])
                            nc.vector.tensor_scalar_mul(vt[nt][:], vt[nt][:], R4)
                            dma(v_o[sl, :], vt[nt][:])
                            nc.vector.tensor_scalar_mul(
                                rho[nt][:], rho[nt][:], AO**nsteps
                            )
                            dma(rate_o[sl, :], rho[nt][:])

                        wo = [wop.tile([128, OUT], F32, tag=f"wo{i}", name=f"wo{i}") for i in range(8)]
                        for j in range(8):
                            dma(wo[j][:], wout_d[128 * j : 128 * (j + 1), :])
                        ps = ppB.tile([128, F], F32, tag="ps", name="ps")
                        for bc in range(2):
                            bsl = slice(512 * bc, 512 * (bc + 1))
                            for j in range(8):
                                nc.tensor.matmul(
                                    ps[:, bsl],
                                    wo[j][:],
                                    rho[j][:, bsl],
                                    start=(j == 0),
                                    stop=(j == 7),
                                )
                        ot = cp.tile([OUT, F], F32, tag="ot", name="ot", bufs=1)
                        nc.scalar.mul(ot[:], ps[:], R_OUT)
                        dma(out_o[:, :], ot[:])

    _split_excess_waits(nc)
    _nc_cache[nsteps] = nc
    return nc


def _prep_in_maps(x, v_m, i_syn, rate, spike):
    f32 = np.float32
    sign = np.ones((N, 1), f32)
    sign[NE:] = -1.0
    wdiff = _prep_in_maps.wdiff
    whs = _prep_in_maps.whs
    wout = _prep_in_maps.wout
    in_maps = []
    for c in range(NCORES):
        rows = slice(BL * c, BL * (c + 1))
        xt = np.ascontiguousarray(x[rows].T).astype(f32)
        xhi = xt.astype(NP_BF16)
        xlo = (xt - xhi.astype(f32)).astype(NP_BF16)
        in_maps.append(
            dict(
                wdiff=wdiff,
                xhi=xhi,
                xlo=xlo,
                whs=whs,
                wout=wout,
                vT=np.ascontiguousarray(v_m[rows].T).astype(f32),
                isynT=np.ascontiguousarray(i_syn[rows].T).astype(f32),
                rateT=np.ascontiguousarray(rate[rows].T).astype(f32),
                spkT=np.ascontiguousarray(np.abs(spike[rows]).T).astype(NP_FP8),
            )
        )
    return in_maps


_runner_cache = {}


def _get_runner(nsteps=NSTEPS):
    """Persistent jitted shard_map executor for the compiled bass program
    (mirrors concourse.bass2jax.run_bass_via_pjrt, but caches the jit so
    repeat calls don't recompile, and exposes device-resident timing)."""
    if nsteps in _runner_cache:
        return _runner_cache[nsteps]
    import jax
    from jax.sharding import Mesh, PartitionSpec
    from jax.experimental.shard_map import shard_map
    from concourse.bass2jax import _bass_exec_p, install_neuronx_cc_hook

    install_neuronx_cc_hook()
    nc = _build_nc(nsteps)
    assert nc.partition_id_tensor is None or True

    in_names, out_names, out_avals = [], [], []
    partition_name = nc.partition_id_tensor.name if nc.partition_id_tensor else None
    for alloc in nc.m.functions[0].allocations:
        if not isinstance(alloc, mybir.MemoryLocationSet):
            continue
        name = alloc.memorylocations[0].name
        if alloc.kind == "ExternalInput":
            if name != partition_name:
                in_names.append(name)
        elif alloc.kind == "ExternalOutput":
            out_names.append(name)
            out_avals.append(
                jax.core.ShapedArray(tuple(alloc.tensor_shape), dt.np(alloc.dtype))
            )
    n_params = len(in_names)
    n_outs = len(out_avals)
    all_in_names = in_names + out_names
    if partition_name is not None:
        all_in_names.append(partition_name)

    def _body(*args):
        operands = list(args)
        if partition_name is not None:
            from concourse.bass2jax import partition_id_tensor

            operands.append(partition_id_tensor())
        outs = _bass_exec_p.bind(
            *operands,
            out_avals=tuple(out_avals),
            in_names=tuple(all_in_names),
            out_names=tuple(out_names),
            lowering_input_output_aliases=(),
            sim_require_finite=True,
            sim_require_nnan=True,
            nc=nc,
        )
        return tuple(outs)

    devices = jax.devices()[:NCORES]
    mesh = Mesh(np.asarray(devices), ("core",))
    donate = tuple(range(n_params, n_params + n_outs))
    sharded = jax.jit(
        shard_map(
            _body,
            mesh=mesh,
            in_specs=(PartitionSpec("core"),) * (n_params + n_outs),
            out_specs=(PartitionSpec("core"),) * n_outs,
            check_rep=False,
        ),
        donate_argnums=donate,
        keep_unused=True,
    )

    zero_shapes = [
        ((NCORES * a.shape[0], *a.shape[1:]), a.dtype) for a in out_avals
    ]
    mk_zeros = jax.jit(
        lambda: tuple(jax.numpy.zeros(s, d) for s, d in zero_shapes),
        out_shardings=tuple(
            jax.sharding.NamedSharding(mesh, PartitionSpec("core"))
            for _ in zero_shapes
        ),
    )

    _runner_cache[nsteps] = dict(
        jax=jax, nc=nc, sharded=sharded, mk_zeros=mk_zeros, mesh=mesh,
        in_names=in_names, out_names=out_names, out_avals=out_avals,
    )
    return _runner_cache[nsteps]


def _exec(in_maps, iters=1, nsteps=NSTEPS):
    """Run the compiled kernel; returns (results_per_core, per_iter_wall_s)."""
    import time as _time

    r = _get_runner(nsteps)
    jax = r["jax"]
    from jax.sharding import NamedSharding, PartitionSpec

    sh = NamedSharding(r["mesh"], PartitionSpec("core"))
    concat_in = [
        jax.device_put(
            np.concatenate([np.asarray(m[n]) for m in in_maps], axis=0), sh
        )
        for n in r["in_names"]
    ]
    jax.block_until_ready(concat_in)

    outs = None
    times = []
    for _ in range(iters):
        zeros = r["mk_zeros"]()
        jax.block_until_ready(zeros)
        t0 = _time.perf_counter()
        outs = r["sharded"](*concat_in, *zeros)
        jax.block_until_ready(outs)
        times.append(_time.perf_counter() - t0)

    results = [
        {
            n: np.asarray(outs[i]).reshape(NCORES, *r["out_avals"][i].shape)[c]
            for i, n in enumerate(r["out_names"])
        }
        for c in range(NCORES)
    ]
    return results, times


def bench_chain(in_maps, k=8, reps=3):
    """Time the kernel by running it k times back-to-back inside one jit,
    chaining v/isyn/rate outputs back into the matching inputs so the NEFF
    executions serialize. Returns per-execution seconds (slope vs k=1)."""
    import time as _time

    r = _get_runner()
    jax = r["jax"]
    import jax.numpy as jnp
    from jax.sharding import NamedSharding, PartitionSpec
    from jax.experimental.shard_map import shard_map
    from concourse.bass2jax import _bass_exec_p

    nc = r["nc"]
    in_names = r["in_names"]
    out_names = r["out_names"]
    out_avals = r["out_avals"]
    partition_name = nc.partition_id_tensor.name if nc.partition_id_tensor else None
    all_in_names = in_names + out_names + (
        [partition_name] if partition_name else []
    )
    feedback = {"vT": "v_o", "isynT": "isyn_o", "rateT": "rate_o"}
    n_in = len(in_names)
    n_out = len(out_names)

    def once(vals, zeros_set):
        operands = [vals[n] for n in in_names] + list(zeros_set)
        if partition_name is not None:
            from concourse.bass2jax import partition_id_tensor

            operands.append(partition_id_tensor())
        outs = _bass_exec_p.bind(
            *operands,
            out_avals=tuple(out_avals),
            in_names=tuple(all_in_names),
            out_names=tuple(out_names),
            lowering_input_output_aliases=(),
            sim_require_finite=True,
            sim_require_nnan=True,
            nc=nc,
        )
        omap = dict(zip(out_names, outs))
        nvals = dict(vals)
        for iname, oname in feedback.items():
            nvals[iname] = omap[oname]
        return nvals, outs

    def make_body(kk):
        def body(*args):
            vals = dict(zip(in_names, args[:n_in]))
            zargs = args[n_in:]
            for i in range(kk):
                vals, outs = once(vals, zargs[i * n_out : (i + 1) * n_out])
            return tuple(outs)

        return body

    sh = NamedSharding(r["mesh"], PartitionSpec("core"))
    concat_in = [
        jax.device_put(
            np.concatenate([np.asarray(m[n]) for m in in_maps], axis=0), sh
        )
        for n in in_names
    ]
    jax.block_until_ready(concat_in)

    zero_shapes = [((NCORES * a.shape[0], *a.shape[1:]), a.dtype) for a in out_avals]
    mk_zeros = jax.jit(
        lambda: tuple(jnp.zeros(s, d) for s, d in zero_shapes),
        out_shardings=tuple(sh for _ in zero_shapes),
    )

    timings = {}
    for kk in (1, k):
        fn = jax.jit(
            shard_map(
                make_body(kk),
                mesh=r["mesh"],
                in_specs=(PartitionSpec("core"),) * (n_in + kk * n_out),
                out_specs=(PartitionSpec("core"),) * n_out,
                check_rep=False,
            ),
            donate_argnums=tuple(range(n_in, n_in + kk * n_out)),
            keep_unused=True,
        )

        def run_once(fn=fn, kk=kk):
            zs = []
            for _ in range(kk):
                zs.extend(mk_zeros())
            jax.block_until_ready(zs)
            t0 = _time.perf_counter()
            outs = fn(*concat_in, *zs)
            jax.block_until_ready(outs)
            return _time.perf_counter() - t0

        run_once()  # compile + warm
        timings[kk] = min(run_once() for _ in range(reps))
    per_exec = (timings[k] - timings[1]) / (k - 1)
    return per_exec, timings


def _run(x, v_m, i_syn, rate, spike, kernel_in, kernel_h, kernel_out, iters=1):
    f32 = np.float32
    ki = np.asarray(kernel_in)
    kh = np.asarray(kernel_h)
    ko = np.asarray(kernel_out)
    sign = np.ones((N, 1), f32)
    sign[NE:] = -1.0
    _prep_in_maps.wdiff = np.ascontiguousarray(
        ki[:IN].astype(np.float32) - ki[IN:].astype(np.float32)
    ).astype(NP_BF16)
    _prep_in_maps.whs = np.ascontiguousarray(kh.astype(f32) * sign).astype(NP_FP8)
    _prep_in_maps.wout = np.ascontiguousarray(ko).astype(f32)

    in_maps = _prep_in_maps(
        np.asarray(x), np.asarray(v_m), np.asarray(i_syn),
        np.asarray(rate), np.asarray(spike),
    )
    results, times = _exec(in_maps, iters=iters)

    v = np.concatenate([results[c]["v_o"].T for c in range(NCORES)], axis=0)
    isyn = np.concatenate([results[c]["isyn_o"].T for c in range(NCORES)], axis=0)
    rate_f = np.concatenate([results[c]["rate_o"].T for c in range(NCORES)], axis=0)
    spk = np.concatenate([results[c]["spk_o"].T for c in range(NCORES)], axis=0)
    out = np.concatenate([results[c]["out_o"].T for c in range(NCORES)], axis=0)
    outputs = (
        v.astype(f32),
        isyn.astype(f32),
        rate_f.astype(f32),
        spk.astype(np.int32),
        out.astype(f32),
    )
    return outputs, times


def kernel(x, v_m, i_syn, rate, spike, kernel_in, kernel_h, kernel_out):
    outputs, _ = _run(x, v_m, i_syn, rate, spike, kernel_in, kernel_h, kernel_out)
    return outputs
